# revision 4
# baseline (speedup 1.0000x reference)
"""Differential cross-attention head on 8 Trainium2 NeuronCores.

Sharding: data-parallel over batch (4) x sequence-parallel over Tq (2) = 8 cores.
Each core computes out[b, h*1024:(h+1)*1024, :] for (b, h) = divmod(core, 2).

Per-core math in "transposed" orientation (no on-chip transposes; host
transposes the output back):
  - qT = Wq^T @ xT            [D, 1024]
  - kT = Wk^T @ encT          [D, Tk]
  - v  = encT^T @ Wv          [Tk, D]
  - s^T = k @ q^T             [Tk, Tq] scores transposed; s1|s2 packed into one
                              [128,1024] PSUM tile via PE row-group tiling
  - e^T = exp(s^T/8)          ScalarE, PSUM->SBUF, bf16
  - A^T += v_chunk^T @ e^T    accumulated in PSUM per group
  - row-sums: DVE/Pool chain accumulation + ones-matmul partition reduce
Normalization out = A1/r1 - lam*A2/r2 and final transpose happen on the host.

Both q-groups are processed per Tk chunk (interleaved) so the Scalar engine
(exp) runs continuously; PSUM = 2 rotating score tiles (4 banks) + 2 PV
accumulators (4 banks) = 8 banks exactly.

DMA: group-major DRAM layouts; weights in one packed tensor; transfers split
across the two HWDGE queues (Sync, Scalar) + GpSimd SWDGE with
dependency-ordered heads so the first attention unit starts as early as
possible.
"""

import sys
from contextlib import ExitStack

import numpy as np

_TRN_REPO = "/opt/trn_rl_repo"
if _TRN_REPO not in sys.path:
    sys.path.insert(0, _TRN_REPO)

import ml_dtypes

import concourse.bass as bass
import concourse.tile as tile
from concourse import mybir
from concourse.bass import ds, ts

F32 = mybir.dt.float32
BF16 = mybir.dt.bfloat16

E = 1024          # embed dim
D = 128           # head dim
B = 4
TQ = 2048
TK = 2048
NCORES = 8
TQL = B * TQ // NCORES   # 1024 query rows per core
EC = E // 128            # 8 contraction chunks for projections
NG = TQL // 512          # 2 query groups of 512
TKG = TK // 512          # 4 Tk groups
KC = TK // 128           # 16 Tk chunks
SCALE = 0.125            # 1/sqrt(64)

NP_BF16 = ml_dtypes.bfloat16


def _build(nc: bass.Bass, with_vbias: bool):
    # x group-major: [g][128 part][EC][512] -> per-partition 8KB linear rows
    xT = nc.dram_tensor("xT", [NG, 128, EC, 512], BF16,
                        kind="ExternalInput").ap()
    encT = nc.dram_tensor("encT", [TKG, 128, EC, 512], BF16,
                          kind="ExternalInput").ap()
    # weights packed [128][3][EC][D]: one linear DMA, 6KB/partition
    wpack = nc.dram_tensor("wpack", [128, 3, EC, D], BF16,
                           kind="ExternalInput").ap()
    bpack = nc.dram_tensor("bpack", [128, 2], F32, kind="ExternalInput").ap()
    bv = nc.dram_tensor("bv", [D], F32, kind="ExternalInput").ap()
    pvd = nc.dram_tensor("pvd", [128, NG * 1024], BF16,
                         kind="ExternalOutput").ap()
    rd = nc.dram_tensor("rd", [NG, 1024], F32, kind="ExternalOutput").ap()

    Exp = mybir.ActivationFunctionType.Exp

    with tile.TileContext(nc) as tc, ExitStack() as ctx:
        const = ctx.enter_context(tc.tile_pool(name="const", bufs=1))
        xpool = ctx.enter_context(tc.tile_pool(name="xpool", bufs=1))
        encpool = ctx.enter_context(tc.tile_pool(name="encpool", bufs=1))
        proj = ctx.enter_context(tc.tile_pool(name="proj", bufs=1))
        epool = ctx.enter_context(tc.tile_pool(name="epool", bufs=6))
        rpool = ctx.enter_context(tc.tile_pool(name="rpool", bufs=6))
        outp = ctx.enter_context(tc.tile_pool(name="outp", bufs=2))
        psS = ctx.enter_context(tc.tile_pool(name="psS", bufs=2, space="PSUM"))
        psPV = ctx.enter_context(tc.tile_pool(name="psPV", bufs=2, space="PSUM"))

        # ---- input DMAs: issue everything up-front, priority-ordered ----
        # sync queue: weights+biases (needed first), then x group 0, then wv
        w3_sb = const.tile([128, 3, EC, D], BF16, tag="w3")
        nc.sync.dma_start(out=w3_sb[:, 0:2], in_=wpack[:, 0:2])   # wq, wk
        b_sb = const.tile([128, 2], F32, tag="b")
        nc.sync.dma_start(out=b_sb, in_=bpack)

        xstage = xpool.tile([128, NG, EC, 512], BF16, tag="xstage")
        for half in range(2):
            nc.sync.dma_start(out=xstage[:, 0, ts(half, 4)],
                              in_=xT[0][:, ts(half, 4)])
        nc.sync.dma_start(out=w3_sb[:, 2:3], in_=wpack[:, 2:3])   # wv
        if with_vbias:
            bv_sb = const.tile([1, D], F32, tag="bv")
            nc.sync.dma_start(out=bv_sb, in_=bv.rearrange("(o d) -> o d", o=1))

        # scalar queue: enc group-by-group (tg0 split for an early k-proj)
        enc_sb = encpool.tile([128, EC, TK], BF16, tag="enc")
        for half in range(2):
            nc.scalar.dma_start(out=enc_sb[:, ts(half, 4), ts(0, 512)],
                                in_=encT[0][:, ts(half, 4)])
        for tg in range(1, TKG):
            nc.scalar.dma_start(out=enc_sb[:, :, ts(tg, 512)], in_=encT[tg])

        # gpsimd SWDGE: x group 1 (third queue)
        nc.gpsimd.dma_start(out=xstage[:, 1], in_=xT[1])

        # constants
        if with_vbias:
            ones_row_f32 = const.tile([1, 128], F32, tag="ones_row_f32")
            nc.vector.memset(ones_row_f32, 1.0)
        ones_col = const.tile([128, 1], BF16, tag="ones_col")
        nc.vector.memset(ones_col, 1.0)

        # ---- q^T projection: qT[D, TQL] = Wq^T @ x^T (+ bq) ----
        qT_sb = proj.tile([128, TQL], BF16, tag="qT")
        for g in range(NG):
            qp = psS.tile([128, 1024], F32, tag="ps_s")
            for c in range(EC):
                nc.tensor.matmul(qp[:, 0:512], lhsT=w3_sb[:, 0, c],
                                 rhs=xstage[:, g, c],
                                 start=(c == 0), stop=(c == EC - 1))
            nc.vector.tensor_scalar_add(qT_sb[:, ts(g, 512)], qp[:, 0:512],
                                        b_sb[:, 0:1])

        kT_sb = proj.tile([128, TK], BF16, tag="kT")
        v_sb = proj.tile([128, KC, D], BF16, tag="v")

        pv = [None, None]
        # row-sum accumulators: 3 chains per group (2 on DVE, 1 on GpSimd)
        racc = [[rpool.tile([128, 1024], BF16, tag="racc", name=f"racc{g}{p}")
                 for p in range(3)] for g in range(NG)]

        def attention_unit(g, k_glob):
            if k_glob == 0:
                pv[g] = psPV.tile([128, 1024], F32, tag="ps_pv", name=f"pv{g}")
            s12 = psS.tile([128, 1024], F32, tag="ps_s", name="s12")
            nc.tensor.matmul(s12[:, 0:512],
                             lhsT=kT_sb[0:64, ts(k_glob, 128)],
                             rhs=qT_sb[0:64, ts(g, 512)],
                             start=True, stop=True, tile_position=(0, 0))
            nc.tensor.matmul(s12[:, 512:1024],
                             lhsT=kT_sb[64:128, ts(k_glob, 128)],
                             rhs=qT_sb[64:128, ts(g, 512)],
                             start=True, stop=True, tile_position=(64, 0))
            e12 = epool.tile([128, 1024], BF16, tag="e", name="e12")
            nc.scalar.activation(e12, s12, Exp, scale=SCALE)
            for h in range(2):
                nc.tensor.matmul(pv[g][:, ts(h, 512)],
                                 lhsT=v_sb[:, k_glob, :],
                                 rhs=e12[:, ts(h, 512)],
                                 start=(k_glob == 0), stop=(k_glob == KC - 1),
                                 skip_group_check=True)
            par = k_glob % 3
            eng = nc.gpsimd if par == 2 else nc.vector
            if k_glob < 3:
                eng.tensor_copy(racc[g][par], e12)
            else:
                eng.tensor_add(racc[g][par], racc[g][par], e12)

        def attention_tail(g):
            nc.vector.tensor_add(racc[g][0], racc[g][0], racc[g][1])
            nc.vector.tensor_add(racc[g][0], racc[g][0], racc[g][2])
            r12p = psS.tile([1, 1024], F32, tag="ps_s", name="r12p")
            for h in range(2):
                nc.tensor.matmul(r12p[:, ts(h, 512)], lhsT=ones_col,
                                 rhs=racc[g][0][:, ts(h, 512)],
                                 start=True, stop=True)
            r_sb = outp.tile([1, 1024], F32, tag="r_sb", name="r_sb")
            nc.vector.tensor_copy(r_sb, r12p)
            pv_sb = outp.tile([128, 1024], BF16, tag="pv_sb", name="pv_sb")
            nc.vector.tensor_copy(pv_sb, pv[g])
            eng = nc.sync if g == 0 else nc.scalar
            eng.dma_start(out=rd[g, :].rearrange("(o t) -> o t", o=1),
                          in_=r_sb)
            eng.dma_start(out=pvd[:, ds(g * 1024, 1024)], in_=pv_sb)

        # ---- main loop: per Tk group project k/v then run both q-groups ----
        for tg in range(TKG):
            # k^T for this Tk group
            kp = psS.tile([128, 1024], F32, tag="ps_s")
            for c in range(EC):
                nc.tensor.matmul(kp[:, 0:512], lhsT=w3_sb[:, 1, c],
                                 rhs=enc_sb[:, c, ts(tg, 512)],
                                 start=(c == 0), stop=(c == EC - 1))
            nc.vector.tensor_scalar_add(kT_sb[:, ts(tg, 512)], kp[:, 0:512],
                                        b_sb[:, 1:2])

            # v (natural) for this group: 4 blocks of [128, 128] in one tile
            vp = psS.tile([128, 1024], F32, tag="ps_s")
            for t in range(4):
                tk = tg * 4 + t
                if with_vbias:
                    nc.tensor.matmul(vp[:, ts(t, 128)], lhsT=ones_row_f32,
                                     rhs=bv_sb, start=True, stop=False,
                                     skip_group_check=True)
                for c in range(EC):
                    nc.tensor.matmul(vp[:, ts(t, 128)],
                                     lhsT=enc_sb[:, c, ts(tk, 128)],
                                     rhs=w3_sb[:, 2, c],
                                     start=(not with_vbias and c == 0),
                                     stop=(c == EC - 1),
                                     skip_group_check=True)
            nc.vector.tensor_copy(
                v_sb[:, tg * 4:(tg + 1) * 4, :].rearrange("p t d -> p (t d)"),
                vp[:, 0:512])

            for kc in range(4):
                k_glob = tg * 4 + kc
                for g in range(NG):
                    attention_unit(g, k_glob)
                if k_glob == KC - 1:
                    attention_tail(0)
                    attention_tail(1)

    return nc


_nc_cache = {}


def _make_bass(with_vbias: bool):
    from concourse import bacc

    nc = bacc.Bacc("TRN2", target_bir_lowering=False, debug=False)
    _build(nc, with_vbias)
    nc.compile()
    return nc


def _pack_x(a):
    """[T=1024, E] -> [NG, 128, EC, 512] bf16 (partition-major per group)."""
    t = a.shape[0]
    # a.T is [E, T]; chunk c rows c*128..., group g cols g*512...
    at = np.ascontiguousarray(a.T.astype(NP_BF16))       # [E, T]
    at = at.reshape(EC, 128, t // 512, 512)              # [c, p, g, 512]
    return np.ascontiguousarray(at.transpose(2, 1, 0, 3))  # [g, p, c, 512]


def _pack_w(W_q, W_k, W_v):
    """3x [E, D] -> [128, 3, EC, D] bf16."""
    w = np.stack([np.asarray(w, np.float32) for w in (W_q, W_k, W_v)])
    w = w.astype(NP_BF16).reshape(3, EC, 128, D)
    return np.ascontiguousarray(w.transpose(2, 0, 1, 3))


def kernel(x, encoder_out, W_q, b_q, W_k, b_k, W_v, b_v,
           lambda_q1, lambda_k1, lambda_q2, lambda_k2, lambda_init):
    from concourse import bass_utils

    x = np.asarray(x, np.float32)
    encoder_out = np.asarray(encoder_out, np.float32)
    wpack = _pack_w(W_q, W_k, W_v)
    bpack = np.ascontiguousarray(
        np.stack([np.asarray(b_q, np.float32),
                  np.asarray(b_k, np.float32)], axis=1))  # [128, 2]
    b_v = np.asarray(b_v, np.float32)

    lam = np.float32(
        np.exp(np.float32(np.asarray(lambda_q1, np.float32)
                          @ np.asarray(lambda_k1, np.float32)))
        - np.exp(np.float32(np.asarray(lambda_q2, np.float32)
                            @ np.asarray(lambda_k2, np.float32)))
        + np.float32(np.asarray(lambda_init, np.float32))
    )

    with_vbias = bool(np.any(b_v))
    if with_vbias not in _nc_cache:
        _nc_cache[with_vbias] = _make_bass(with_vbias)
    nc = _nc_cache[with_vbias]

    encTs = [_pack_x(encoder_out[b]) for b in range(B)]  # [TKG,128,EC,512]
    in_maps = []
    for c in range(NCORES):
        b, h = divmod(c, 2)
        xTs = _pack_x(x[b, h * TQL:(h + 1) * TQL, :])
        in_maps.append({
            "xT": xTs, "encT": encTs[b],
            "wpack": wpack, "bpack": bpack, "bv": b_v,
        })

    res = bass_utils.run_bass_kernel_spmd(nc, in_maps, core_ids=list(range(NCORES)))
    kernel.last_result = res

    out = np.empty((B, TQ, D), np.float32)
    for c in range(NCORES):
        b, h = divmod(c, 2)
        pvd = np.asarray(res.results[c]["pvd"], np.float32)  # [D, NG*1024]
        rd = np.asarray(res.results[c]["rd"], np.float32)    # [NG, 1024]
        for g in range(NG):
            A = pvd[:, g * 1024:(g + 1) * 1024]
            A1, A2 = A[:, 0:512], A[:, 512:1024]
            r1, r2 = rd[g, 0:512], rd[g, 512:1024]
            o = A1 / r1 - lam * (A2 / r2)    # [D, 512]
            q0 = h * TQL + g * 512
            out[b, q0:q0 + 512, :] = o.T
    return out
